# revision 38
# baseline (speedup 1.0000x reference)
"""Trainium2 Bass kernel for nn_Attention_5583457485032 (8 NeuronCores).

Reference (per head i of 2, W[i] is (256,256), iterated twice over
x (200000, 256)):
    temp = mean(xi, 0);  h = tanh(temp @ Wi);  s = xi @ h.T
    att = sigmoid(s / max(|s|, 1e-12));  out = att.T @ xi;  xi = xi * att
returns concat of head outputs, shape (1, 512).

Algebraic restructuring (exact):
  - att == sigmoid(sign(s)) == a- + D*u with u = [s > 0],
    a- = sigmoid(-1), D = sigmoid(1)-sigmoid(-1)
  - out1 = a-*cs + D*S1      with cs = colsum(x), S1 = sum_{u1} x
  - out2 = a-^2*cs + a-*D*(S1+S2) + D^2*S12,  S2 = sum_{u2} x,
    S12 = sum_{u1*u2} x
  The device computes cs, S1, S2, S12 per core; the host combines in
  f64.  Pad rows (x=0) are exactly neutral in every sum.

Host prep: x is cast to bf16, padded to 196 full 128-row tiles and
rearranged tile-major per core so the load DMA is fully contiguous
(4 KB per partition line).  W is cast to bf16.

Kernel phases:
  A: HWDGE (sync-engine) chunk DMAs stream x into resident x_nat.
     PE transposes each 128x128 tile half with a [ones|iden] moving
     operand (129 cols): col 0 of each output is the tile-half colsum
     (f32 psum).  DVE accumulates the colsum columns (1 add/tile) and
     splits the psum->xts bf16 copies with ACT.  colsum -> AllReduce.
  B: scores via xts-stationary matmuls (both heads), u1 = is_gt on
     DVE, out matmuls accumulate S1.  a-*cs@W is pre-folded into the
     h2 psum during B.  S1 -> AllReduce (raw).
  C: h2 = tanh((a-*cs@W + D*S1@W)/N); same pass shape with
     stationaries [u2, u1*u2] accumulating S2,S12.
A dummy collective is triggered at program start so the CC channel
bootstrap overlaps the load.

Raw Bass with a two-pass scheduler: pass 1 plans semaphore values for
every instruction, pass 2 emits per-engine programs with deduped
wait_ge()s.
"""

import os
import numpy as np

N_CORES = 8
N_TOTAL = 200000
D = 256
H = 2
P = 128
TPC = 8   # tiles per DMA chunk
G = 16    # tiles per score/u group

AM = 1.0 / (1.0 + float(np.exp(1.0)))    # sigmoid(-1)
APOS = 1.0 / (1.0 + float(np.exp(-1.0)))  # sigmoid(1)
DL = APOS - AM


def build_kernel(n_rows_pad, n_cores, n_total):
    import concourse.bass as bass
    import concourse.mybir as mybir

    F32 = mybir.dt.float32
    BF16 = mybir.dt.bfloat16
    AF = mybir.ActivationFunctionType
    ALU = mybir.AluOpType

    assert n_rows_pad % P == 0
    T = n_rows_pad // P
    n_chunks = (T + TPC - 1) // TPC
    chunk_tiles = [list(range(c * TPC, min(T, (c + 1) * TPC)))
                   for c in range(n_chunks)]
    n_groups = (T + G - 1) // G
    group_tiles = [list(range(g * G, min(T, (g + 1) * G)))
                   for g in range(n_groups)]
    warm = os.environ.get("NO_WARM", "") == ""

    nc = bass.Bass()
    x_ext = nc.declare_dram_parameter("x", [P, T * D], BF16, isOutput=False)
    w_ext = nc.declare_dram_parameter("W", [H, D, D], BF16, isOutput=False)
    out_ext = nc.declare_dram_parameter("out", [6, D], F32, isOutput=True)
    cs_ext = nc.declare_dram_parameter("cs", [P, 2], F32, isOutput=True)
    dbg_ext = nc.declare_dram_parameter("dbg", [P, 4 * H], F32, isOutput=True)

    cs_dram = nc.dram_tensor("cs_dram", [P, 2], F32)
    cs_ar = nc.dram_tensor("cs_ar", [P, 2], F32)
    o1_dram = nc.dram_tensor("o1_dram", [H, D], F32)
    o1_ar = nc.dram_tensor("o1_ar", [H, D], F32)
    warm_d = nc.dram_tensor("warm_d", [2, 2], F32)
    warm_o = nc.dram_tensor("warm_o", [2, 2], F32)

    sb = nc.alloc_sbuf_tensor
    x_nat = sb("x_nat", [P, T * D], BF16)   # tile t at cols [t*D,(t+1)*D)
    xts = sb("xts", [P, T * D], BF16)       # tile t: [t*D + oc*P + row]
    iden_b = sb("iden_b", [P, P], BF16)     # 128x128 identity
    ones_b = sb("ones_b", [P, P], BF16)
    ones_f = sb("ones_f", [2, 2], F32)
    if2 = sb("if2", [2, 2], F32)            # 2x2 f32 identity (h2 transpose)
    wsb = sb("wsb", [P, H * 2 * 2 * P], BF16)  # block (h,dc,oc) at ((h*2+dc)*2+oc)*128
    hcol1 = sb("hcol1", [P, 2 * H], BF16)   # [oc*H + h]
    hcol2 = sb("hcol2", [P, 2 * H], BF16)
    cs_stage = sb("cs_stage", [P, 512], F32)  # per-tile colsums [t][oc], tree-reduced
    cs_g = sb("cs_g", [P, 2], F32)          # AR1 result
    cs_colb = sb("cs_colb", [P, 2], BF16)
    csb_am = sb("csb_am", [P, 2], BF16)     # a- * cs_g, bf16
    att1 = sb("att1", [P, ((T + G - 1) // G) * G * H], BF16)  # u1, padded to full groups
    att2 = [sb(f"att2{b}", [P, G * 4], BF16) for b in range(3)]
    S1_sb = sb("S1_sb", [H, D], F32)
    S1r = sb("S1r", [H, D], F32)            # AR2 result
    s1b = sb("s1b", [H, D], F32)            # D * S1r (f32: transpose dtype)
    ztb = sb("ztb", [P, 2 * H], BF16)       # transposed D*S1, [dc*H+h]
    o_sb = sb("o_sb", [4, D], F32)
    dbg_sb = sb("dbg_sb", [P, 4 * H], F32)

    # PSUM map: 8 tensors = 8 banks.  A psum bank must never be read by
    # DVE/ACT while the PE is concurrently writing ANY region of it
    # (same-bank read/write wedges the HW), so the three score slots get
    # their own banks (the PE writes slot g+1/g+2 while DVE reads slot g).
    ps = nc.alloc_psum_tensor
    xtp = [ps(f"xtp{b}", [P, D], BF16) for b in range(3)]  # bf16 transpose slots
    spsum = [ps(f"sp{b}", [P, G * H], F32).ap() for b in range(3)]
    # zt, ht1, ht2 share one f32 bank.  ht1 is dead before anything else
    # writes the bank; the h2 transposes into zt use start=False (their
    # region is pending-zero, so the lazy clear yields correct values)
    # to avoid re-marking the bank and losing ht2's prefold accumulation.
    zzh = ps("zzh", [P, 512], F32)
    zt = zzh.ap()[:, 0:2 * H]
    ht1 = zzh.ap()[:, 128:128 + 2 * H]
    ht2 = zzh.ap()[:, 256:256 + 2 * H]
    S12p = ps("S12p", [2 * H, 512], F32)   # S1p | S2p in one bank
    S1p = S12p.ap()[0:H, 0:D]
    S2p = S12p.ap()[0:2 * H, D:2 * D]

    sems = {k: nc.alloc_semaphore(k) for k in
            ("dma_w", "dma_x0", "dma_x1", "dma_x2", "dma_x3", "dma_x4",
             "dma_x5", "dma_x6", "dma_x7", "dma_m", "dma_p",
             "pe", "act", "dve", "cc", "poolc")}

    ENGS = ("sp", "pe", "act", "dve", "pool")

    class Sched:
        def __init__(self, plan=None):
            self.plan = plan
            self.ctr = {k: 0 for k in sems}
            self.ev = {} if plan is None else plan
            self.ops = {e: [] for e in ENGS}
            self.seen = {e: {} for e in ENGS}

        def inst(self, eng, sem, thunk, key=None, step=1):
            self.ctr[sem] += step
            v = self.ctr[sem]
            if self.plan is None:
                if key is not None:
                    assert key not in self.ev, key
                    self.ev[key] = (sem, v)
            else:
                if key is not None:
                    assert self.ev[key] == (sem, v), (key, self.ev[key], sem, v)
                self.ops[eng].append(("i", thunk, sem, step))
            return v

        def wait(self, eng, key):
            if self.plan is None:
                return
            sem, v = self.ev[key]
            if v <= 0 or self.seen[eng].get(sem, 0) >= v:
                return
            self.seen[eng][sem] = v
            self.ops[eng].append(("w", sem, v))

    meta = {"eng_of": {}}

    def sched(S):
        # ---- preamble ----
        for _b in range(3):
            S.inst("dve", "dve",
                   lambda _b=_b: nc.vector.memset(spsum[_b], 0.0))
        S.inst("dve", "dve", lambda: nc.vector.memset(cs_stage.ap(), 0.0),
               key=("dve", "cs0"))
        S.inst("pool", "poolc", lambda: nc.gpsimd.memset(ones_b.ap(), 1.0),
               key=("pool", "ones"))
        S.wait("pool", ("pool", "ones"))
        S.inst("pool", "poolc",
               lambda: nc.gpsimd.affine_select(
                   iden_b.ap(), ones_b.ap(), pattern=[[-1, P]],
                   compare_op=ALU.is_equal, fill=0.0, base=0,
                   channel_multiplier=1),
               key=("pool", "io"))
        S.inst("pool", "poolc", lambda: nc.gpsimd.memset(ones_f.ap(), 1.0))
        S.inst("pool", "poolc",
               lambda: nc.gpsimd.affine_select(
                   if2.ap(), ones_f.ap(), pattern=[[-1, 2]],
                   compare_op=ALU.is_equal, fill=0.0, base=0,
                   channel_multiplier=1),
               key=("pool", "if2"))
        if warm:
            S.wait("sp", ("dve", "cs0"))
            S.inst("sp", "dma_m",
                   lambda: nc.sync.dma_start(out=warm_d[:, :],
                                             in_=cs_stage[0:2, 508:510]),
                   step=16, key=("dma", "warm"))
            S.wait("pool", ("dma", "warm"))
            S.inst("pool", "cc",
                   lambda: nc.gpsimd.collective_compute(
                       "AllReduce", mybir.AluOpType.add,
                       replica_groups=[list(range(n_cores))],
                       ins=[warm_d[:, :]], outs=[warm_o[:, :]]),
                   key=("cc", "warm"))
        # W load: 4 DMAs, one per (h, dc) row-block (natural layout)
        w_eng = os.environ.get("W_ENG", "pool")
        w_dma = {"sp": nc.sync, "pool": nc.gpsimd}[w_eng]
        for h in range(H):
            for dc in range(2):
                base = (h * 2 + dc) * 2 * P
                S.inst(w_eng, "dma_w",
                       lambda h=h, dc=dc, base=base, w_dma=w_dma:
                       w_dma.dma_start(
                           out=wsb[:, base:base + 2 * P],
                           in_=w_ext[h, dc * P:(dc + 1) * P, :]),
                       step=16,
                       key=("dma", "W") if (h, dc) == (H - 1, 1) else None)

        # ---- phase A: chunk loads + PE transposes ----
        load_eng = os.environ.get("LOAD_ENG", "sp")
        for c in range(n_chunks):
            sem = f"dma_x{c % 8}"
            if c >= 8:
                S.wait(load_eng, ("dma", "load", c - 8))
            c0 = c * TPC * D
            c1 = min(T, (c + 1) * TPC) * D
            dma_of = {"sp": nc.sync, "pool": nc.gpsimd, "act": nc.scalar}[load_eng]
            S.inst(load_eng, sem,
                   lambda c0=c0, c1=c1, dma_of=dma_of:
                   dma_of.dma_start(out=x_nat[:, c0:c1], in_=x_ext[:, c0:c1]),
                   step=16, key=("dma", "load", c))

        def emit_tr(t):
            if t >= 3:
                if int(os.environ.get("STRIP", "9")) >= 3:
                    S.wait("pe", ("cp", "A_copy", t - 3))
            if t == 0:
                S.wait("pe", ("pool", "io"))
            S.wait("pe", ("dma", "load", t // TPC))
            for oc in range(2):
                S.inst("pe", "pe",
                       lambda t=t, oc=oc:
                       nc.tensor.transpose(
                           xtp[t % 3].ap()[:, oc * P:(oc + 1) * P],
                           x_nat[:, t * D + oc * P:t * D + (oc + 1) * P],
                           iden_b[:, :]),
                       key=("pe", "A_tr", t) if oc == 1 else None)

        def emit_copy(t):
            # per-half psum->xts copies (plain 2-D APs); accum_out emits
            # the per-partition row-sum = exact tile-half colsum (the
            # copied values are bf16-exact, so the f32 accum is exact).
            # Small direct PSUM reads on DVE wedge the HW; accum_out on a
            # wide copy avoids them.
            use_dve = t % 3 != 2
            meta["eng_of"][t] = "dve" if use_dve else "act"
            eng, sem = ("dve", "dve") if use_dve else ("act", "act")
            S.wait(eng, ("pe", "A_tr", t))
            for oc in range(2):
                dst = xts.ap()[:, t * D + oc * P:t * D + oc * P + P]
                src = xtp[t % 3].ap()[:, oc * P:oc * P + P]
                acc = cs_stage.ap()[:, 2 * t + oc:2 * t + oc + 1]
                if use_dve:
                    S.inst(eng, sem,
                           lambda dst=dst, src=src, acc=acc:
                           nc.vector.tensor_scalar(
                               dst, src, 0.0, 0.0, ALU.add, ALU.add,
                               accum_out=acc),
                           key=("cp", "A_copy", t) if oc == 1 else None)
                else:
                    S.inst(eng, sem,
                           lambda dst=dst, src=src, acc=acc:
                           nc.scalar.activation(
                               dst, src, AF.Copy, accum_out=acc),
                           key=("cp", "A_copy", t) if oc == 1 else None)

        strip = int(os.environ.get("STRIP", "9"))
        for c in range(n_chunks):
            for t in chunk_tiles[c]:
                if strip >= 2:
                    emit_tr(t)
            for t in chunk_tiles[c]:
                if strip >= 3:
                    emit_copy(t)

        # tree-reduce cs_stage (256 tile slots, zero-padded) -> [:, 0:2].
        # Same-engine in-place chain: explicit self-waits between levels.
        lvl = 0
        if strip >= 5:
            S.wait("dve", ("cp", "A_copy", T - 1))
            S.wait("dve", ("cp", "A_copy", T - 2))
            k = 256
            while k > 1:
                k //= 2
                S.inst("dve", "dve",
                       lambda k=k: nc.vector.tensor_add(
                           cs_stage.ap()[:, 0:2 * k],
                           cs_stage.ap()[:, 0:2 * k],
                           cs_stage.ap()[:, 2 * k:4 * k]),
                       key=("dve", "tree", lvl))
                S.wait("dve", ("dve", "tree", lvl))
                lvl += 1

        if os.environ.get("PHASES", "full") == "a0":
            # loads+transposes+copies+tree only; no collectives at all
            S.inst("dve", "dve", lambda: nc.vector.memset(S1_sb.ap(), 0.0),
                   key=("dve", "stub1"))
            S.inst("dve", "dve", lambda: nc.vector.memset(o_sb.ap(), 0.0),
                   key=("dve", "stub2"))
            S.wait("sp", ("dve", "stub2"))
            if lvl > 0:
                S.wait("sp", ("dve", "tree", lvl - 1))
            S.inst("sp", "dma_m",
                   lambda: nc.sync.dma_start(out=out_ext[0:H, :],
                                             in_=S1_sb[0:H, :]), step=16)
            S.inst("sp", "dma_m",
                   lambda: nc.sync.dma_start(out=out_ext[H:6, :],
                                             in_=o_sb[0:4, :]), step=16)
            S.inst("sp", "dma_m",
                   lambda: nc.sync.dma_start(out=cs_ext[:, :],
                                             in_=cs_stage[:, 0:2]),
                   step=16, key=("dma", "out_final"))
            S.wait("sp", ("dma", "out_final"))
            return

        # colsum -> AR1
        S.wait("sp", ("dve", "tree", lvl - 1))
        S.inst("sp", "dma_m",
               lambda: nc.sync.dma_start(out=cs_dram[:, :],
                                         in_=cs_stage[:, 0:2]),
               step=16, key=("dma", "cs_out"))
        S.wait("pool", ("dma", "cs_out"))
        S.inst("pool", "cc",
               lambda: nc.gpsimd.collective_compute(
                   "AllReduce", mybir.AluOpType.add,
                   replica_groups=[list(range(n_cores))],
                   ins=[cs_dram[:, :]], outs=[cs_ar[:, :]]),
               key=("cc", "ar1"))

        # ---- h1 ----
        S.wait("sp", ("cc", "ar1"))
        S.inst("sp", "dma_m",
               lambda: nc.sync.dma_start(out=cs_g[:, :], in_=cs_ar[:, :]),
               step=16, key=("dma", "h1_in"))
        S.wait("dve", ("dma", "h1_in"))
        S.inst("dve", "dve",
               lambda: nc.vector.tensor_copy(cs_colb[:, :], cs_g[:, :]),
               key=("dve", "h1_colb"))
        S.inst("dve", "dve",
               lambda: nc.vector.tensor_scalar_mul(csb_am[:, :], cs_g[:, :], AM),
               key=("dve", "csb_am"))
        S.wait("pe", ("dma", "W"))
        S.wait("pe", ("dve", "h1_colb"))
        for h in range(H):
            for oc in range(2):
                for dc in range(2):
                    widx = (h * 2 + dc) * 2 + oc
                    S.inst("pe", "pe",
                           lambda h=h, oc=oc, dc=dc, widx=widx:
                           nc.tensor.matmul(
                               ht1[:, oc * H + h:oc * H + h + 1],
                               wsb[:, widx * P:(widx + 1) * P],
                               cs_colb[:, dc:dc + 1],
                               start=(dc == 0), stop=(dc == 1),
                               skip_group_check=True),
                           key=("pe", "h1_mm")
                           if (h, oc, dc) == (H - 1, 1, 1) else None)
        S.wait("act", ("pe", "h1_mm"))
        S.inst("act", "act",
               lambda: nc.scalar.activation(
                   hcol1[:, :], ht1[:, :], AF.Tanh, scale=1.0 / float(n_total)),
               key=("act", "h1"))

        # ---- passes B and C ----
        def pass_bc(tag):
            is_c = tag == "C"
            hcol = hcol2 if is_c else hcol1
            htag = "h2" if is_c else "h1"

            def mms_tile(t):
                g = t // G
                b = g % 3
                col = (t - g * G) * H
                if t == 0:
                    S.wait("pe", ("act", htag))
                if not is_c and t == group_tiles[g][0]:
                    for engname in ("dve", "act"):
                        owned = [tt for tt in group_tiles[g]
                                 if meta["eng_of"].get(tt) == engname]
                        if owned:
                            S.wait("pe", ("cp", "A_copy", max(owned)))
                if g >= 3 and t == group_tiles[g][0]:
                    S.wait("pe", ("dve", tag + "_u", g - 3))
                for oc in range(2):
                    S.inst("pe", "pe",
                           lambda t=t, b=b, col=col, oc=oc, hcol=hcol:
                           nc.tensor.matmul(
                               spsum[b][:, col:col + H],
                               xts[:, t * D + oc * P:t * D + oc * P + P],
                               hcol[:, oc * H:(oc + 1) * H],
                               start=(oc == 0), stop=(oc == 1),
                               skip_group_check=True),
                           key=("pe", tag + "_mmS", t) if oc == 1 else None)

            def mmout_tile(t):
                g = t // G
                if is_c:
                    S.wait("pe", ("dve", "C_w", g))
                    lhs = lambda t=t, g=g: att2[g % 3][:, (t - g * G) * 4:
                                                       (t - g * G) * 4 + 4]
                    dst = S2p
                else:
                    S.wait("pe", ("dve", "B_u", g))
                    lhs = lambda t=t: att1[:, t * H:(t + 1) * H]
                    dst = S1p
                S.inst("pe", "pe",
                       lambda t=t, lhs=lhs, dst=dst:
                       nc.tensor.matmul(
                           dst[:, :],
                           lhs(),
                           x_nat[:, t * D:(t + 1) * D],
                           start=(t == 0), stop=(t == T - 1),
                           skip_group_check=True),
                       key=("pe", tag + "_mmOut", t))

            def u_group(g):
                # always full-G rectangles: narrow DVE PSUM reads (< ~16
                # cols) wedge the hardware; padded att1/att2 regions
                # absorb the garbage columns of partial groups.
                b = g % 3
                ncols = G * H
                S.wait("dve", ("pe", tag + "_mmS", group_tiles[g][-1]))
                sp_view = spsum[b][:, 0:ncols].rearrange(
                    "p (t h) -> p t h", h=H)
                if is_c:
                    if g >= 3:
                        # att2 slot g%3 must be done being read by the
                        # out-matmuls of group g-3 before overwriting
                        S.wait("dve", ("pe", "C_mmOut",
                                       group_tiles[g - 3][-1]))
                    a2 = att2[g % 3].ap()[:, 0:G * 4]
                    a2v = a2.rearrange("p (t c) -> p t c", c=4)
                    S.inst("dve", "dve",
                           lambda g=g, sp_view=sp_view, a2v=a2v:
                           nc.vector.tensor_scalar(
                               a2v[:, :, 0:2], sp_view, 0.0, None,
                               ALU.is_gt),
                           key=("dve", "C_u", g))
                    at1 = att1.ap()[:, g * G * H:g * G * H + ncols].rearrange(
                        "p (t h) -> p t h", h=H)
                    S.wait("dve", ("dve", "C_u", g))  # same-engine RAW
                    S.inst("dve", "dve",
                           lambda g=g, a2v=a2v, at1=at1:
                           nc.vector.tensor_mul(
                               a2v[:, :, 2:4], a2v[:, :, 0:2], at1),
                           key=("dve", "C_w", g))
                else:
                    S.inst("dve", "dve",
                           lambda g=g, ncols=ncols, sp_view=sp_view:
                           nc.vector.tensor_scalar(
                               att1.ap()[:, g * G * H:g * G * H + ncols]
                               .rearrange("p (t h) -> p t h", h=H),
                               sp_view, 0.0, None, ALU.is_gt),
                           key=("dve", "B_u", g))

            for g0 in range(min(2, n_groups)):
                for t in group_tiles[g0]:
                    mms_tile(t)
                if g0 == 0 and not is_c:
                    # prefold a-*cs@W into ht2 while B runs.  start=True
                    # marks the WHOLE psum bank row pending-zero (lazy
                    # bank-granular clear), so only the first matmul may
                    # carry it -- later columns' first writes consume
                    # their pending-zero bytes and then accumulate.
                    S.wait("pe", ("dve", "csb_am"))
                    first = [True]
                    for h in range(H):
                        for oc in range(2):
                            for dc in range(2):
                                widx = (h * 2 + dc) * 2 + oc
                                st = first[0]
                                first[0] = False
                                S.inst("pe", "pe",
                                       lambda h=h, oc=oc, dc=dc, widx=widx,
                                       st=st:
                                       nc.tensor.matmul(
                                           ht2[:, oc * H + h:oc * H + h + 1],
                                           wsb[:, widx * P:(widx + 1) * P],
                                           csb_am[:, dc:dc + 1],
                                           start=st, stop=False,
                                           skip_group_check=True))
            for g in range(n_groups):
                if g + 2 < n_groups:
                    for t in group_tiles[g + 2]:
                        mms_tile(t)
                for t in group_tiles[g]:
                    mmout_tile(t)
            for g in range(n_groups):
                u_group(g)

        phases = os.environ.get("PHASES", "full")
        if phases == "a":
            S.inst("dve", "dve", lambda: nc.vector.memset(S1_sb.ap(), 0.0),
                   key=("dve", "stub1"))
            S.inst("dve", "dve", lambda: nc.vector.memset(o_sb.ap(), 0.0),
                   key=("dve", "stub2"))
            S.wait("sp", ("dve", "stub2"))
            S.inst("sp", "dma_m",
                   lambda: nc.sync.dma_start(out=out_ext[0:H, :],
                                             in_=S1_sb[0:H, :]), step=16)
            S.inst("sp", "dma_m",
                   lambda: nc.sync.dma_start(out=out_ext[H:6, :],
                                             in_=o_sb[0:4, :]), step=16)
            S.inst("sp", "dma_m",
                   lambda: nc.sync.dma_start(out=cs_ext[:, :],
                                             in_=cs_stage[:, 0:2]),
                   step=16, key=("dma", "out_final"))
            S.wait("sp", ("dma", "out_final"))
            return

        pass_bc("B")

        if phases == "ab":
            S.wait("act", ("pe", "B_mmOut", T - 1))
            S.inst("act", "act",
                   lambda: nc.scalar.copy(S1_sb[0:H, :], S1p[:, :]),
                   key=("act", "o1_copy"))
            S.inst("dve", "dve", lambda: nc.vector.memset(o_sb.ap(), 0.0),
                   key=("dve", "stub2"))
            S.wait("sp", ("act", "o1_copy"))
            S.wait("sp", ("dve", "stub2"))
            S.inst("sp", "dma_m",
                   lambda: nc.sync.dma_start(out=out_ext[0:H, :],
                                             in_=S1_sb[0:H, :]), step=16)
            S.inst("sp", "dma_m",
                   lambda: nc.sync.dma_start(out=out_ext[H:6, :],
                                             in_=o_sb[0:4, :]), step=16)
            S.inst("sp", "dma_m",
                   lambda: nc.sync.dma_start(out=cs_ext[:, :],
                                             in_=cs_stage[:, 0:2]),
                   step=16, key=("dma", "out_final"))
            S.wait("sp", ("dma", "out_final"))
            return

        # S1 -> AR2 (raw partials)
        S.wait("act", ("pe", "B_mmOut", T - 1))
        S.inst("act", "act",
               lambda: nc.scalar.copy(S1_sb[0:H, :], S1p[:, :]),
               key=("act", "o1_copy"))
        S.wait("sp", ("act", "o1_copy"))
        S.inst("sp", "dma_m",
               lambda: nc.sync.dma_start(out=o1_dram[:, :], in_=S1_sb[0:H, :]),
               step=16, key=("dma", "o1_out"))
        S.wait("pool", ("dma", "o1_out"))
        S.inst("pool", "cc",
               lambda: nc.gpsimd.collective_compute(
                   "AllReduce", mybir.AluOpType.add,
                   replica_groups=[list(range(n_cores))],
                   ins=[o1_dram[:, :]], outs=[o1_ar[:, :]]),
               key=("cc", "ar2"))

        # ---- h2 ----
        S.wait("sp", ("cc", "ar2"))
        S.inst("sp", "dma_m",
               lambda: nc.sync.dma_start(out=S1r[0:H, :], in_=o1_ar[:, :]),
               step=16, key=("dma", "h2_in"))
        S.wait("dve", ("dma", "h2_in"))
        S.inst("dve", "dve",
               lambda: nc.vector.tensor_scalar_mul(s1b[0:H, :], S1r[0:H, :], DL),
               key=("dve", "s1b"))
        S.wait("pe", ("dve", "s1b"))
        S.wait("pe", ("pool", "if2"))
        for dc in range(2):
            S.inst("pe", "pe",
                   lambda dc=dc:
                   nc.tensor.matmul(
                       zt[:, dc * H:(dc + 1) * H],
                       s1b[0:H, dc * P:(dc + 1) * P],
                       if2[0:H, 0:H],
                       start=False, stop=True,
                       is_transpose=True,
                       skip_group_check=True),
                   key=("pe", "h2_tr") if dc == 1 else None)
        S.wait("dve", ("pe", "h2_tr"))
        S.inst("dve", "dve",
               lambda: nc.vector.tensor_copy(ztb[:, :], zt[:, :]),
               key=("dve", "h2_col"))
        S.wait("pe", ("dve", "h2_col"))
        for h in range(H):
            for oc in range(2):
                for dc in range(2):
                    widx = (h * 2 + dc) * 2 + oc
                    S.inst("pe", "pe",
                           lambda h=h, oc=oc, dc=dc, widx=widx:
                           nc.tensor.matmul(
                               ht2[:, oc * H + h:oc * H + h + 1],
                               wsb[:, widx * P:(widx + 1) * P],
                               ztb[:, dc * H + h:dc * H + h + 1],
                               start=False,
                               stop=(dc == 1),
                               skip_group_check=True),
                           key=("pe", "h2_mm")
                           if (h, oc, dc) == (H - 1, 1, 1) else None)
        S.wait("act", ("pe", "h2_mm"))
        S.inst("act", "act",
               lambda: nc.scalar.activation(
                   hcol2[:, :], ht2[:, :], AF.Tanh, scale=1.0 / float(n_total)),
               key=("act", "h2"))

        pass_bc("C")

        # ---- final outputs ----
        S.wait("act", ("pe", "C_mmOut", T - 1))
        S.inst("act", "act",
               lambda: nc.scalar.copy(o_sb[0:4, :], S2p[:, :]),
               key=("act", "out_copy"))
        S.wait("sp", ("act", "out_copy"))
        S.inst("sp", "dma_m",
               lambda: nc.sync.dma_start(out=out_ext[0:H, :], in_=S1_sb[0:H, :]),
               step=16)
        S.inst("sp", "dma_m",
               lambda: nc.sync.dma_start(out=out_ext[H:6, :], in_=o_sb[0:4, :]),
               step=16)
        S.inst("dve", "dve",
               lambda: nc.vector.tensor_copy(dbg_sb[:, 0:2 * H], hcol1[:, :]),
               key=("dve", "dbg1"))
        S.inst("dve", "dve",
               lambda: nc.vector.tensor_copy(dbg_sb[:, 2 * H:4 * H], hcol2[:, :]),
               key=("dve", "dbg2"))
        S.wait("sp", ("dve", "dbg2"))
        S.inst("sp", "dma_m",
               lambda: nc.sync.dma_start(out=dbg_ext[:, :], in_=dbg_sb[:, :]),
               step=16)
        S.inst("sp", "dma_m",
               lambda: nc.sync.dma_start(out=cs_ext[:, :],
                                         in_=cs_stage[:, 0:2]),
               step=16, key=("dma", "out_final"))
        S.wait("sp", ("dma", "out_final"))

    plan = Sched()
    sched(plan)
    emit = Sched(plan.ev)
    sched(emit)

    eng_map = {
        "sp": nc.sync, "pe": nc.tensor, "act": nc.scalar,
        "dve": nc.vector, "pool": nc.gpsimd,
    }

    def run_ops(eng_name):
        eng = eng_map[eng_name]
        def body(_engine):
            for op in emit.ops[eng_name]:
                if op[0] == "w":
                    _, sem, v = op
                    eng.wait_ge(sems[sem], v)
                else:
                    _, thunk, sem, step = op
                    bi = thunk()
                    bi.then_inc(sems[sem], step)
        return body

    with nc.Block() as block:
        block.sync(run_ops("sp"))
        block.gpsimd(run_ops("pool"))
        block.tensor(run_ops("pe"))
        block.scalar(run_ops("act"))
        block.vector(run_ops("dve"))

    return nc


_NC_CACHE = {}


def _get_nc(n_rows_pad, n_cores, n_total):
    key = (n_rows_pad, n_cores, n_total)
    if key not in _NC_CACHE:
        _NC_CACHE[key] = build_kernel(n_rows_pad, n_cores, n_total)
    return _NC_CACHE[key]


def prep_inputs(x, W):
    """Host-side: cast to bf16, pad to full tiles, tile-major layout."""
    import ml_dtypes
    x = np.asarray(x, dtype=np.float32)
    W = np.asarray(W, dtype=np.float32)
    n, d = x.shape
    assert n % N_CORES == 0 and d == D
    n_rows = n // N_CORES
    T = (n_rows + P - 1) // P
    n_pad = T * P
    xb = x.astype(ml_dtypes.bfloat16)
    Wb = np.ascontiguousarray(W.astype(ml_dtypes.bfloat16))
    in_maps = []
    for c in range(N_CORES):
        xs = xb[c * n_rows:(c + 1) * n_rows]
        if n_pad != n_rows:
            xs = np.concatenate(
                [xs, np.zeros((n_pad - n_rows, d), dtype=ml_dtypes.bfloat16)])
        tiled = np.ascontiguousarray(
            xs.reshape(T, P, d).transpose(1, 0, 2).reshape(P, T * d))
        in_maps.append({"x": tiled, "W": Wb})
    return in_maps, n_pad


def combine(results):
    """Host-side: sum per-core partials and apply the exact formula."""
    S = np.zeros((6, D), dtype=np.float64)
    CS = np.zeros((P, 2), dtype=np.float64)
    for r in results:
        S += r["out"].astype(np.float64)
        CS += r["cs"].astype(np.float64)
    cs_row = CS.T.reshape(D)            # feat = oc*128 + p
    outs = []
    for h in range(H):
        S1 = S[h]
        S2 = S[2 + h]
        S12 = S[4 + h]
        out2 = AM * AM * cs_row + AM * DL * (S1 + S2) + DL * DL * S12
        outs.append(out2)
    return np.concatenate(outs).reshape(1, H * D).astype(np.float32)


def kernel(x, W):
    from concourse.bass_utils import run_bass_kernel_spmd

    in_maps, n_pad = prep_inputs(x, W)
    nc = _get_nc(n_pad, N_CORES, x.shape[0])
    res = run_bass_kernel_spmd(nc, in_maps, core_ids=list(range(N_CORES)))
    return combine(res.results)


if __name__ == "__main__":
    rng = np.random.default_rng(0)
    x = rng.standard_normal((N_TOTAL, D)).astype(np.float32)
    W = (rng.standard_normal((H, D, D)) * np.sqrt(2.0 / (D + D))).astype(np.float32)
    out = kernel(x=x, W=W)
    print(out.shape, out[0, :4])


# revision 40
# speedup vs baseline: 1.0213x; 1.0213x over previous
"""Trainium2 Bass kernel for nn_Attention_5583457485032 (8 NeuronCores).

Reference (per head i of 2, W[i] is (256,256), iterated twice over
x (200000, 256)):
    temp = mean(xi, 0);  h = tanh(temp @ Wi);  s = xi @ h.T
    att = sigmoid(s / max(|s|, 1e-12));  out = att.T @ xi;  xi = xi * att
returns concat of head outputs, shape (1, 512).

Algebraic restructuring (exact):
  - att == sigmoid(sign(s)) == a- + D*u with u = [s > 0],
    a- = sigmoid(-1), D = sigmoid(1)-sigmoid(-1)
  - out1 = a-*cs + D*S1      with cs = colsum(x), S1 = sum_{u1} x
  - out2 = a-^2*cs + a-*D*(S1+S2) + D^2*S12,  S2 = sum_{u2} x,
    S12 = sum_{u1*u2} x
  The device computes cs, S1, S2, S12 per core; the host combines in
  f64.  Pad rows (x=0) are exactly neutral in every sum.

Host prep: x is cast to bf16, padded to 196 full 128-row tiles and
rearranged tile-major per core so the load DMA is fully contiguous
(4 KB per partition line).  W is cast to bf16.

Kernel phases:
  A: HWDGE (sync-engine) chunk DMAs stream x into resident x_nat.
     PE transposes each 128x128 tile half with a [ones|iden] moving
     operand (129 cols): col 0 of each output is the tile-half colsum
     (f32 psum).  DVE accumulates the colsum columns (1 add/tile) and
     splits the psum->xts bf16 copies with ACT.  colsum -> AllReduce.
  B: scores via xts-stationary matmuls (both heads), u1 = is_gt on
     DVE, out matmuls accumulate S1.  a-*cs@W is pre-folded into the
     h2 psum during B.  S1 -> AllReduce (raw).
  C: h2 = tanh((a-*cs@W + D*S1@W)/N); same pass shape with
     stationaries [u2, u1*u2] accumulating S2,S12.
A dummy collective is triggered at program start so the CC channel
bootstrap overlaps the load.

Raw Bass with a two-pass scheduler: pass 1 plans semaphore values for
every instruction, pass 2 emits per-engine programs with deduped
wait_ge()s.
"""

import os
import numpy as np

N_CORES = 8
N_TOTAL = 200000
D = 256
H = 2
P = 128
TPC = 8   # tiles per DMA chunk
G = 16    # tiles per score/u group

AM = 1.0 / (1.0 + float(np.exp(1.0)))    # sigmoid(-1)
APOS = 1.0 / (1.0 + float(np.exp(-1.0)))  # sigmoid(1)
DL = APOS - AM


def build_kernel(n_rows_pad, n_cores, n_total):
    import concourse.bass as bass
    import concourse.mybir as mybir

    F32 = mybir.dt.float32
    BF16 = mybir.dt.bfloat16
    AF = mybir.ActivationFunctionType
    ALU = mybir.AluOpType

    assert n_rows_pad % P == 0
    T = n_rows_pad // P
    n_chunks = (T + TPC - 1) // TPC
    chunk_tiles = [list(range(c * TPC, min(T, (c + 1) * TPC)))
                   for c in range(n_chunks)]
    n_groups = (T + G - 1) // G
    group_tiles = [list(range(g * G, min(T, (g + 1) * G)))
                   for g in range(n_groups)]
    warm = os.environ.get("WARM", "") != ""

    nc = bass.Bass()
    x_ext = nc.declare_dram_parameter("x", [P, T * D], BF16, isOutput=False)
    w_ext = nc.declare_dram_parameter("W", [H, D, D], BF16, isOutput=False)
    out_ext = nc.declare_dram_parameter("out", [6, D], F32, isOutput=True)
    cs_ext = nc.declare_dram_parameter("cs", [P, 2], F32, isOutput=True)
    dbg_ext = nc.declare_dram_parameter("dbg", [P, 4 * H], F32, isOutput=True)

    cs_dram = nc.dram_tensor("cs_dram", [P, 2], F32)
    cs_ar = nc.dram_tensor("cs_ar", [P, 2], F32)
    o1_dram = nc.dram_tensor("o1_dram", [H, D], F32)
    o1_ar = nc.dram_tensor("o1_ar", [H, D], F32)
    warm_d = nc.dram_tensor("warm_d", [2, 2], F32)
    warm_o = nc.dram_tensor("warm_o", [2, 2], F32)

    sb = nc.alloc_sbuf_tensor
    x_nat = sb("x_nat", [P, T * D], BF16)   # tile t at cols [t*D,(t+1)*D)
    xts = sb("xts", [P, T * D], BF16)       # tile t: [t*D + oc*P + row]
    iden_b = sb("iden_b", [P, P], BF16)     # 128x128 identity
    ones_b = sb("ones_b", [P, P], BF16)
    ones_f = sb("ones_f", [2, 2], F32)
    if2 = sb("if2", [2, 2], F32)            # 2x2 f32 identity (h2 transpose)
    wsb = sb("wsb", [P, H * 2 * 2 * P], BF16)  # block (h,dc,oc) at ((h*2+dc)*2+oc)*128
    hcol1 = sb("hcol1", [P, 2 * H], BF16)   # [oc*H + h]
    hcol2 = sb("hcol2", [P, 2 * H], BF16)
    cs_stage = sb("cs_stage", [P, 512], F32)  # per-tile colsums [t][oc], tree-reduced
    cs_g = sb("cs_g", [P, 2], F32)          # AR1 result
    cs_colb = sb("cs_colb", [P, 2], BF16)
    csb_am = sb("csb_am", [P, 2], BF16)     # a- * cs_g, bf16
    att1 = sb("att1", [P, ((T + G - 1) // G) * G * H], BF16)  # u1, padded to full groups
    att2 = [sb(f"att2{b}", [P, G * 4], BF16) for b in range(3)]
    S1_sb = sb("S1_sb", [H, D], F32)
    S1r = sb("S1r", [H, D], F32)            # AR2 result
    s1b = sb("s1b", [H, D], F32)            # D * S1r (f32: transpose dtype)
    ztb = sb("ztb", [P, 2 * H], BF16)       # transposed D*S1, [dc*H+h]
    o_sb = sb("o_sb", [4, D], F32)
    dbg_sb = sb("dbg_sb", [P, 4 * H], F32)

    # PSUM map: 8 tensors = 8 banks.  A psum bank must never be read by
    # DVE/ACT while the PE is concurrently writing ANY region of it
    # (same-bank read/write wedges the HW), so the three score slots get
    # their own banks (the PE writes slot g+1/g+2 while DVE reads slot g).
    ps = nc.alloc_psum_tensor
    xtp = [ps(f"xtp{b}", [P, 4 * D], BF16) for b in range(3)]  # 4-tile bf16 slots
    spsum = [ps(f"sp{b}", [P, G * H], F32).ap() for b in range(3)]
    # zt, ht1, ht2 share one f32 bank.  ht1 is dead before anything else
    # writes the bank; the h2 transposes into zt use start=False (their
    # region is pending-zero, so the lazy clear yields correct values)
    # to avoid re-marking the bank and losing ht2's prefold accumulation.
    zzh = ps("zzh", [P, 512], F32)
    zt = zzh.ap()[:, 0:2 * H]
    ht1 = zzh.ap()[:, 128:128 + 2 * H]
    ht2 = zzh.ap()[:, 256:256 + 2 * H]
    S12p = ps("S12p", [2 * H, 512], F32)   # S1p | S2p in one bank
    S1p = S12p.ap()[0:H, 0:D]
    S2p = S12p.ap()[0:2 * H, D:2 * D]

    sems = {k: nc.alloc_semaphore(k) for k in
            ("dma_w", "dma_x0", "dma_x1", "dma_x2", "dma_x3", "dma_x4",
             "dma_x5", "dma_x6", "dma_x7", "dma_m", "dma_p",
             "pe", "act", "dve", "cc", "poolc")}

    ENGS = ("sp", "pe", "act", "dve", "pool")

    class Sched:
        def __init__(self, plan=None):
            self.plan = plan
            self.ctr = {k: 0 for k in sems}
            self.ev = {} if plan is None else plan
            self.ops = {e: [] for e in ENGS}
            self.seen = {e: {} for e in ENGS}

        def inst(self, eng, sem, thunk, key=None, step=1):
            self.ctr[sem] += step
            v = self.ctr[sem]
            if self.plan is None:
                if key is not None:
                    assert key not in self.ev, key
                    self.ev[key] = (sem, v)
            else:
                if key is not None:
                    assert self.ev[key] == (sem, v), (key, self.ev[key], sem, v)
                self.ops[eng].append(("i", thunk, sem, step))
            return v

        def wait(self, eng, key):
            if self.plan is None:
                return
            sem, v = self.ev[key]
            if v <= 0 or self.seen[eng].get(sem, 0) >= v:
                return
            self.seen[eng][sem] = v
            self.ops[eng].append(("w", sem, v))

    meta = {"eng_of": {}}

    def sched(S):
        # ---- preamble ----
        for _b in range(3):
            S.inst("dve", "dve",
                   lambda _b=_b: nc.vector.memset(spsum[_b], 0.0))
        S.inst("dve", "dve", lambda: nc.vector.memset(cs_stage.ap(), 0.0),
               key=("dve", "cs0"))
        S.inst("pool", "poolc", lambda: nc.gpsimd.memset(ones_b.ap(), 1.0),
               key=("pool", "ones"))
        S.wait("pool", ("pool", "ones"))
        S.inst("pool", "poolc",
               lambda: nc.gpsimd.affine_select(
                   iden_b.ap(), ones_b.ap(), pattern=[[-1, P]],
                   compare_op=ALU.is_equal, fill=0.0, base=0,
                   channel_multiplier=1),
               key=("pool", "io"))
        S.inst("pool", "poolc", lambda: nc.gpsimd.memset(ones_f.ap(), 1.0))
        S.inst("pool", "poolc",
               lambda: nc.gpsimd.affine_select(
                   if2.ap(), ones_f.ap(), pattern=[[-1, 2]],
                   compare_op=ALU.is_equal, fill=0.0, base=0,
                   channel_multiplier=1),
               key=("pool", "if2"))
        if warm:
            S.wait("sp", ("dve", "cs0"))
            S.inst("sp", "dma_m",
                   lambda: nc.sync.dma_start(out=warm_d[:, :],
                                             in_=cs_stage[0:2, 508:510]),
                   step=16, key=("dma", "warm"))
            S.wait("pool", ("dma", "warm"))
            S.inst("pool", "cc",
                   lambda: nc.gpsimd.collective_compute(
                       "AllReduce", mybir.AluOpType.add,
                       replica_groups=[list(range(n_cores))],
                       ins=[warm_d[:, :]], outs=[warm_o[:, :]]),
                   key=("cc", "warm"))
        # W load: 4 DMAs, one per (h, dc) row-block (natural layout)
        w_eng = os.environ.get("W_ENG", "pool")
        w_dma = {"sp": nc.sync, "pool": nc.gpsimd}[w_eng]
        for h in range(H):
            for dc in range(2):
                base = (h * 2 + dc) * 2 * P
                S.inst(w_eng, "dma_w",
                       lambda h=h, dc=dc, base=base, w_dma=w_dma:
                       w_dma.dma_start(
                           out=wsb[:, base:base + 2 * P],
                           in_=w_ext[h, dc * P:(dc + 1) * P, :]),
                       step=16,
                       key=("dma", "W") if (h, dc) == (H - 1, 1) else None)

        # ---- phase A: chunk loads + PE transposes ----
        load_eng = os.environ.get("LOAD_ENG", "sp")
        for c in range(n_chunks):
            sem = f"dma_x{c % 8}"
            if c >= 8:
                S.wait(load_eng, ("dma", "load", c - 8))
            c0 = c * TPC * D
            c1 = min(T, (c + 1) * TPC) * D
            dma_of = {"sp": nc.sync, "pool": nc.gpsimd, "act": nc.scalar}[load_eng]
            S.inst(load_eng, sem,
                   lambda c0=c0, c1=c1, dma_of=dma_of:
                   dma_of.dma_start(out=x_nat[:, c0:c1], in_=x_ext[:, c0:c1]),
                   step=16, key=("dma", "load", c))

        def emit_tr(t):
            g4 = t // 4
            if g4 >= 3 and t % 4 == 0:
                S.wait("pe", ("cpg", g4 - 3))
            if t == 0:
                S.wait("pe", ("pool", "io"))
            S.wait("pe", ("dma", "load", t // TPC))
            for oc in range(2):
                S.inst("pe", "pe",
                       lambda t=t, oc=oc:
                       nc.tensor.transpose(
                           xtp[(t // 4) % 3].ap()[
                               :, (t % 4) * D + oc * P:(t % 4) * D + (oc + 1) * P],
                           x_nat[:, t * D + oc * P:t * D + (oc + 1) * P],
                           iden_b[:, :]),
                       key=("pe", "A_tr", t) if oc == 1 else None)

        def emit_copy_group(g4):
            # copy a full 4-tile slot group after its last transpose; the
            # copying engine must never read a bank the PE still writes.
            b = g4 % 3
            use_dve = b != 2
            eng, sem = ("dve", "dve") if use_dve else ("act", "act")
            t_hi = g4 * 4 + 3
            S.wait(eng, ("pe", "A_tr", t_hi))
            for ti in range(4):
                t = g4 * 4 + ti
                for oc in range(2):
                    dst = xts.ap()[:, t * D + oc * P:t * D + oc * P + P]
                    src = xtp[b].ap()[:, ti * D + oc * P:ti * D + oc * P + P]
                    acc = cs_stage.ap()[:, 2 * t + oc:2 * t + oc + 1]
                    last = ti == 3 and oc == 1
                    if use_dve:
                        S.inst(eng, sem,
                               lambda dst=dst, src=src, acc=acc:
                               nc.vector.tensor_scalar(
                                   dst, src, 0.0, 0.0, ALU.add, ALU.add,
                                   accum_out=acc),
                               key=("cpg", g4) if last else None)
                    else:
                        S.inst(eng, sem,
                               lambda dst=dst, src=src, acc=acc:
                               nc.scalar.activation(
                                   dst, src, AF.Copy, accum_out=acc),
                               key=("cpg", g4) if last else None)

        assert T % 4 == 0 and TPC % 4 == 0
        for c in range(n_chunks):
            for t in chunk_tiles[c]:
                emit_tr(t)
            for t in chunk_tiles[c]:
                if t % 4 == 3:
                    emit_copy_group(t // 4)

        # tree-reduce cs_stage (256 tile slots, zero-padded) -> [:, 0:2].
        # Same-engine in-place chain: explicit self-waits between levels.
        lvl = 0
        if True:
            S.wait("dve", ("cpg", T // 4 - 1))
            S.wait("dve", ("cpg", T // 4 - 2))
            S.wait("dve", ("cpg", T // 4 - 3))
            k = 256
            while k > 1:
                k //= 2
                S.inst("dve", "dve",
                       lambda k=k: nc.vector.tensor_add(
                           cs_stage.ap()[:, 0:2 * k],
                           cs_stage.ap()[:, 0:2 * k],
                           cs_stage.ap()[:, 2 * k:4 * k]),
                       key=("dve", "tree", lvl))
                S.wait("dve", ("dve", "tree", lvl))
                lvl += 1

        if os.environ.get("PHASES", "full") == "a0":
            # loads+transposes+copies+tree only; no collectives at all
            S.inst("dve", "dve", lambda: nc.vector.memset(S1_sb.ap(), 0.0),
                   key=("dve", "stub1"))
            S.inst("dve", "dve", lambda: nc.vector.memset(o_sb.ap(), 0.0),
                   key=("dve", "stub2"))
            S.wait("sp", ("dve", "stub2"))
            if lvl > 0:
                S.wait("sp", ("dve", "tree", lvl - 1))
            S.inst("sp", "dma_m",
                   lambda: nc.sync.dma_start(out=out_ext[0:H, :],
                                             in_=S1_sb[0:H, :]), step=16)
            S.inst("sp", "dma_m",
                   lambda: nc.sync.dma_start(out=out_ext[H:6, :],
                                             in_=o_sb[0:4, :]), step=16)
            S.inst("sp", "dma_m",
                   lambda: nc.sync.dma_start(out=cs_ext[:, :],
                                             in_=cs_stage[:, 0:2]),
                   step=16, key=("dma", "out_final"))
            S.wait("sp", ("dma", "out_final"))
            return

        # colsum -> AR1
        S.wait("sp", ("dve", "tree", lvl - 1))
        S.inst("sp", "dma_m",
               lambda: nc.sync.dma_start(out=cs_dram[:, :],
                                         in_=cs_stage[:, 0:2]),
               step=16, key=("dma", "cs_out"))
        S.wait("pool", ("dma", "cs_out"))
        S.inst("pool", "cc",
               lambda: nc.gpsimd.collective_compute(
                   "AllReduce", mybir.AluOpType.add,
                   replica_groups=[list(range(n_cores))],
                   ins=[cs_dram[:, :]], outs=[cs_ar[:, :]]),
               key=("cc", "ar1"))

        # ---- h1 ----
        S.wait("sp", ("cc", "ar1"))
        S.inst("sp", "dma_m",
               lambda: nc.sync.dma_start(out=cs_g[:, :], in_=cs_ar[:, :]),
               step=16, key=("dma", "h1_in"))
        S.wait("dve", ("dma", "h1_in"))
        S.inst("dve", "dve",
               lambda: nc.vector.tensor_copy(cs_colb[:, :], cs_g[:, :]),
               key=("dve", "h1_colb"))
        S.inst("dve", "dve",
               lambda: nc.vector.tensor_scalar_mul(csb_am[:, :], cs_g[:, :], AM),
               key=("dve", "csb_am"))
        S.wait("pe", ("dma", "W"))
        S.wait("pe", ("dve", "h1_colb"))
        for h in range(H):
            for oc in range(2):
                for dc in range(2):
                    widx = (h * 2 + dc) * 2 + oc
                    S.inst("pe", "pe",
                           lambda h=h, oc=oc, dc=dc, widx=widx:
                           nc.tensor.matmul(
                               ht1[:, oc * H + h:oc * H + h + 1],
                               wsb[:, widx * P:(widx + 1) * P],
                               cs_colb[:, dc:dc + 1],
                               start=(dc == 0), stop=(dc == 1),
                               skip_group_check=True),
                           key=("pe", "h1_mm")
                           if (h, oc, dc) == (H - 1, 1, 1) else None)
        S.wait("act", ("pe", "h1_mm"))
        S.inst("act", "act",
               lambda: nc.scalar.activation(
                   hcol1[:, :], ht1[:, :], AF.Tanh, scale=1.0 / float(n_total)),
               key=("act", "h1"))

        # ---- passes B and C ----
        def pass_bc(tag):
            is_c = tag == "C"
            hcol = hcol2 if is_c else hcol1
            htag = "h2" if is_c else "h1"

            def mms_tile(t):
                g = t // G
                b = g % 3
                col = (t - g * G) * H
                if t == 0:
                    S.wait("pe", ("act", htag))
                if not is_c and t == group_tiles[g][0]:
                    for g4 in range(g * G // 4, min((g * G + G) // 4, T // 4)):
                        S.wait("pe", ("cpg", g4))
                if g >= 3 and t == group_tiles[g][0]:
                    S.wait("pe", ("dve", tag + "_u", g - 3))
                for oc in range(2):
                    S.inst("pe", "pe",
                           lambda t=t, b=b, col=col, oc=oc, hcol=hcol:
                           nc.tensor.matmul(
                               spsum[b][:, col:col + H],
                               xts[:, t * D + oc * P:t * D + oc * P + P],
                               hcol[:, oc * H:(oc + 1) * H],
                               start=(oc == 0), stop=(oc == 1),
                               skip_group_check=True),
                           key=("pe", tag + "_mmS", t) if oc == 1 else None)

            def mmout_tile(t):
                g = t // G
                if is_c:
                    S.wait("pe", ("dve", "C_w", g))
                    lhs = lambda t=t, g=g: att2[g % 3][:, (t - g * G) * 4:
                                                       (t - g * G) * 4 + 4]
                    dst = S2p
                else:
                    S.wait("pe", ("dve", "B_u", g))
                    lhs = lambda t=t: att1[:, t * H:(t + 1) * H]
                    dst = S1p
                S.inst("pe", "pe",
                       lambda t=t, lhs=lhs, dst=dst:
                       nc.tensor.matmul(
                           dst[:, :],
                           lhs(),
                           x_nat[:, t * D:(t + 1) * D],
                           start=(t == 0), stop=(t == T - 1),
                           skip_group_check=True),
                       key=("pe", tag + "_mmOut", t))

            def u_group(g):
                # always full-G rectangles: narrow DVE PSUM reads (< ~16
                # cols) wedge the hardware; padded att1/att2 regions
                # absorb the garbage columns of partial groups.
                b = g % 3
                ncols = G * H
                S.wait("dve", ("pe", tag + "_mmS", group_tiles[g][-1]))
                sp_view = spsum[b][:, 0:ncols].rearrange(
                    "p (t h) -> p t h", h=H)
                if is_c:
                    if g >= 3:
                        # att2 slot g%3 must be done being read by the
                        # out-matmuls of group g-3 before overwriting
                        S.wait("dve", ("pe", "C_mmOut",
                                       group_tiles[g - 3][-1]))
                    a2 = att2[g % 3].ap()[:, 0:G * 4]
                    a2v = a2.rearrange("p (t c) -> p t c", c=4)
                    S.inst("dve", "dve",
                           lambda g=g, sp_view=sp_view, a2v=a2v:
                           nc.vector.tensor_scalar(
                               a2v[:, :, 0:2], sp_view, 0.0, None,
                               ALU.is_gt),
                           key=("dve", "C_u", g))
                    at1 = att1.ap()[:, g * G * H:g * G * H + ncols].rearrange(
                        "p (t h) -> p t h", h=H)
                    S.wait("dve", ("dve", "C_u", g))  # same-engine RAW
                    S.inst("dve", "dve",
                           lambda g=g, a2v=a2v, at1=at1:
                           nc.vector.tensor_mul(
                               a2v[:, :, 2:4], a2v[:, :, 0:2], at1),
                           key=("dve", "C_w", g))
                else:
                    S.inst("dve", "dve",
                           lambda g=g, ncols=ncols, sp_view=sp_view:
                           nc.vector.tensor_scalar(
                               att1.ap()[:, g * G * H:g * G * H + ncols]
                               .rearrange("p (t h) -> p t h", h=H),
                               sp_view, 0.0, None, ALU.is_gt),
                           key=("dve", "B_u", g))

            for g0 in range(min(2, n_groups)):
                for t in group_tiles[g0]:
                    mms_tile(t)
                if g0 == 0 and not is_c:
                    # prefold a-*cs@W into ht2 while B runs.  start=True
                    # marks the WHOLE psum bank row pending-zero (lazy
                    # bank-granular clear), so only the first matmul may
                    # carry it -- later columns' first writes consume
                    # their pending-zero bytes and then accumulate.
                    S.wait("pe", ("dve", "csb_am"))
                    first = [True]
                    for h in range(H):
                        for oc in range(2):
                            for dc in range(2):
                                widx = (h * 2 + dc) * 2 + oc
                                st = first[0]
                                first[0] = False
                                S.inst("pe", "pe",
                                       lambda h=h, oc=oc, dc=dc, widx=widx,
                                       st=st:
                                       nc.tensor.matmul(
                                           ht2[:, oc * H + h:oc * H + h + 1],
                                           wsb[:, widx * P:(widx + 1) * P],
                                           csb_am[:, dc:dc + 1],
                                           start=st, stop=False,
                                           skip_group_check=True))
            for g in range(n_groups):
                if g + 2 < n_groups:
                    for t in group_tiles[g + 2]:
                        mms_tile(t)
                for t in group_tiles[g]:
                    mmout_tile(t)
            for g in range(n_groups):
                u_group(g)

        phases = os.environ.get("PHASES", "full")
        if phases == "a":
            S.inst("dve", "dve", lambda: nc.vector.memset(S1_sb.ap(), 0.0),
                   key=("dve", "stub1"))
            S.inst("dve", "dve", lambda: nc.vector.memset(o_sb.ap(), 0.0),
                   key=("dve", "stub2"))
            S.wait("sp", ("dve", "stub2"))
            S.inst("sp", "dma_m",
                   lambda: nc.sync.dma_start(out=out_ext[0:H, :],
                                             in_=S1_sb[0:H, :]), step=16)
            S.inst("sp", "dma_m",
                   lambda: nc.sync.dma_start(out=out_ext[H:6, :],
                                             in_=o_sb[0:4, :]), step=16)
            S.inst("sp", "dma_m",
                   lambda: nc.sync.dma_start(out=cs_ext[:, :],
                                             in_=cs_stage[:, 0:2]),
                   step=16, key=("dma", "out_final"))
            S.wait("sp", ("dma", "out_final"))
            return

        pass_bc("B")

        if phases == "ab":
            S.wait("act", ("pe", "B_mmOut", T - 1))
            S.inst("act", "act",
                   lambda: nc.scalar.copy(S1_sb[0:H, :], S1p[:, :]),
                   key=("act", "o1_copy"))
            S.inst("dve", "dve", lambda: nc.vector.memset(o_sb.ap(), 0.0),
                   key=("dve", "stub2"))
            S.wait("sp", ("act", "o1_copy"))
            S.wait("sp", ("dve", "stub2"))
            S.inst("sp", "dma_m",
                   lambda: nc.sync.dma_start(out=out_ext[0:H, :],
                                             in_=S1_sb[0:H, :]), step=16)
            S.inst("sp", "dma_m",
                   lambda: nc.sync.dma_start(out=out_ext[H:6, :],
                                             in_=o_sb[0:4, :]), step=16)
            S.inst("sp", "dma_m",
                   lambda: nc.sync.dma_start(out=cs_ext[:, :],
                                             in_=cs_stage[:, 0:2]),
                   step=16, key=("dma", "out_final"))
            S.wait("sp", ("dma", "out_final"))
            return

        # S1 -> AR2 (raw partials)
        S.wait("act", ("pe", "B_mmOut", T - 1))
        S.inst("act", "act",
               lambda: nc.scalar.copy(S1_sb[0:H, :], S1p[:, :]),
               key=("act", "o1_copy"))
        S.wait("sp", ("act", "o1_copy"))
        S.inst("sp", "dma_m",
               lambda: nc.sync.dma_start(out=o1_dram[:, :], in_=S1_sb[0:H, :]),
               step=16, key=("dma", "o1_out"))
        S.wait("pool", ("dma", "o1_out"))
        S.inst("pool", "cc",
               lambda: nc.gpsimd.collective_compute(
                   "AllReduce", mybir.AluOpType.add,
                   replica_groups=[list(range(n_cores))],
                   ins=[o1_dram[:, :]], outs=[o1_ar[:, :]]),
               key=("cc", "ar2"))

        # ---- h2 ----
        S.wait("sp", ("cc", "ar2"))
        S.inst("sp", "dma_m",
               lambda: nc.sync.dma_start(out=S1r[0:H, :], in_=o1_ar[:, :]),
               step=16, key=("dma", "h2_in"))
        S.wait("dve", ("dma", "h2_in"))
        S.inst("dve", "dve",
               lambda: nc.vector.tensor_scalar_mul(s1b[0:H, :], S1r[0:H, :], DL),
               key=("dve", "s1b"))
        S.wait("pe", ("dve", "s1b"))
        S.wait("pe", ("pool", "if2"))
        for dc in range(2):
            S.inst("pe", "pe",
                   lambda dc=dc:
                   nc.tensor.matmul(
                       zt[:, dc * H:(dc + 1) * H],
                       s1b[0:H, dc * P:(dc + 1) * P],
                       if2[0:H, 0:H],
                       start=False, stop=True,
                       is_transpose=True,
                       skip_group_check=True),
                   key=("pe", "h2_tr") if dc == 1 else None)
        S.wait("dve", ("pe", "h2_tr"))
        S.inst("dve", "dve",
               lambda: nc.vector.tensor_copy(ztb[:, :], zt[:, :]),
               key=("dve", "h2_col"))
        S.wait("pe", ("dve", "h2_col"))
        for h in range(H):
            for oc in range(2):
                for dc in range(2):
                    widx = (h * 2 + dc) * 2 + oc
                    S.inst("pe", "pe",
                           lambda h=h, oc=oc, dc=dc, widx=widx:
                           nc.tensor.matmul(
                               ht2[:, oc * H + h:oc * H + h + 1],
                               wsb[:, widx * P:(widx + 1) * P],
                               ztb[:, dc * H + h:dc * H + h + 1],
                               start=False,
                               stop=(dc == 1),
                               skip_group_check=True),
                           key=("pe", "h2_mm")
                           if (h, oc, dc) == (H - 1, 1, 1) else None)
        S.wait("act", ("pe", "h2_mm"))
        S.inst("act", "act",
               lambda: nc.scalar.activation(
                   hcol2[:, :], ht2[:, :], AF.Tanh, scale=1.0 / float(n_total)),
               key=("act", "h2"))

        pass_bc("C")

        # ---- final outputs ----
        S.wait("act", ("pe", "C_mmOut", T - 1))
        S.inst("act", "act",
               lambda: nc.scalar.copy(o_sb[0:4, :], S2p[:, :]),
               key=("act", "out_copy"))
        S.wait("sp", ("act", "out_copy"))
        S.inst("sp", "dma_m",
               lambda: nc.sync.dma_start(out=out_ext[0:H, :], in_=S1_sb[0:H, :]),
               step=16)
        S.inst("sp", "dma_m",
               lambda: nc.sync.dma_start(out=out_ext[H:6, :], in_=o_sb[0:4, :]),
               step=16)
        if os.environ.get("DBG", ""):
            S.inst("dve", "dve",
                   lambda: nc.vector.tensor_copy(dbg_sb[:, 0:2 * H], hcol1[:, :]),
                   key=("dve", "dbg1"))
            S.inst("dve", "dve",
                   lambda: nc.vector.tensor_copy(dbg_sb[:, 2 * H:4 * H],
                                                 hcol2[:, :]),
                   key=("dve", "dbg2"))
            S.wait("sp", ("dve", "dbg2"))
            S.inst("sp", "dma_m",
                   lambda: nc.sync.dma_start(out=dbg_ext[:, :],
                                             in_=dbg_sb[:, :]),
                   step=16)
        S.inst("sp", "dma_m",
               lambda: nc.sync.dma_start(out=cs_ext[:, :],
                                         in_=cs_stage[:, 0:2]),
               step=16, key=("dma", "out_final"))
        S.wait("sp", ("dma", "out_final"))

    plan = Sched()
    sched(plan)
    emit = Sched(plan.ev)
    sched(emit)

    eng_map = {
        "sp": nc.sync, "pe": nc.tensor, "act": nc.scalar,
        "dve": nc.vector, "pool": nc.gpsimd,
    }

    def run_ops(eng_name):
        eng = eng_map[eng_name]
        def body(_engine):
            for op in emit.ops[eng_name]:
                if op[0] == "w":
                    _, sem, v = op
                    eng.wait_ge(sems[sem], v)
                else:
                    _, thunk, sem, step = op
                    bi = thunk()
                    bi.then_inc(sems[sem], step)
        return body

    with nc.Block() as block:
        block.sync(run_ops("sp"))
        block.gpsimd(run_ops("pool"))
        block.tensor(run_ops("pe"))
        block.scalar(run_ops("act"))
        block.vector(run_ops("dve"))

    return nc


_NC_CACHE = {}


def _get_nc(n_rows_pad, n_cores, n_total):
    key = (n_rows_pad, n_cores, n_total)
    if key not in _NC_CACHE:
        _NC_CACHE[key] = build_kernel(n_rows_pad, n_cores, n_total)
    return _NC_CACHE[key]


def prep_inputs(x, W):
    """Host-side: cast to bf16, pad to full tiles, tile-major layout."""
    import ml_dtypes
    x = np.asarray(x, dtype=np.float32)
    W = np.asarray(W, dtype=np.float32)
    n, d = x.shape
    assert n % N_CORES == 0 and d == D
    n_rows = n // N_CORES
    T = (n_rows + P - 1) // P
    n_pad = T * P
    xb = x.astype(ml_dtypes.bfloat16)
    Wb = np.ascontiguousarray(W.astype(ml_dtypes.bfloat16))
    in_maps = []
    for c in range(N_CORES):
        xs = xb[c * n_rows:(c + 1) * n_rows]
        if n_pad != n_rows:
            xs = np.concatenate(
                [xs, np.zeros((n_pad - n_rows, d), dtype=ml_dtypes.bfloat16)])
        tiled = np.ascontiguousarray(
            xs.reshape(T, P, d).transpose(1, 0, 2).reshape(P, T * d))
        in_maps.append({"x": tiled, "W": Wb})
    return in_maps, n_pad


def combine(results):
    """Host-side: sum per-core partials and apply the exact formula."""
    S = np.zeros((6, D), dtype=np.float64)
    CS = np.zeros((P, 2), dtype=np.float64)
    for r in results:
        S += r["out"].astype(np.float64)
        CS += r["cs"].astype(np.float64)
    cs_row = CS.T.reshape(D)            # feat = oc*128 + p
    outs = []
    for h in range(H):
        S1 = S[h]
        S2 = S[2 + h]
        S12 = S[4 + h]
        out2 = AM * AM * cs_row + AM * DL * (S1 + S2) + DL * DL * S12
        outs.append(out2)
    return np.concatenate(outs).reshape(1, H * D).astype(np.float32)


def kernel(x, W):
    from concourse.bass_utils import run_bass_kernel_spmd

    in_maps, n_pad = prep_inputs(x, W)
    nc = _get_nc(n_pad, N_CORES, x.shape[0])
    res = run_bass_kernel_spmd(nc, in_maps, core_ids=list(range(N_CORES)))
    return combine(res.results)


if __name__ == "__main__":
    rng = np.random.default_rng(0)
    x = rng.standard_normal((N_TOTAL, D)).astype(np.float32)
    W = (rng.standard_normal((H, D, D)) * np.sqrt(2.0 / (D + D))).astype(np.float32)
    out = kernel(x=x, W=W)
    print(out.shape, out[0, :4])
